# revision 1
# baseline (speedup 1.0000x reference)
"""Trainium2 Bass kernel for nn_Distribution_74758200754679.

Computes, for x [65536, 8, 256] and a tiny MLP (256 -> 128 -> 1):
    h    = leaky_relu(x @ W1 + b1, 0.3)
    beta = sigmoid(h @ W2 + b2)            # [B, N]
    p    = stick_breaking(beta)            # [B, N+1]

Distribution: pure data parallel over 8 NeuronCores — x is sharded along
the batch axis, MLP params are replicated. Each core's shard is staged
host-side in transposed layout (d_in on partitions) so the device loop is
a straight chain of full-rate matmuls with no on-chip transpose.

Per-core device program (64 MB of x per core, 128 blocks x 512 rows):
  DMA xT block chunks -> PE fp32r matmuls (L1, accumulate K=256 in PSUM)
  -> ACT 0.7*relu(z+b1) -> DVE hh = 0.3*z + r (leaky) -> PE L2 matmul
  -> DVE copy [1,512] -> tiny DMA gathers beta rows into [block, row] tile
  -> tail: sigmoid + suffix-product stick-breaking, one contiguous DMA out.
"""

import os
import sys

# The device path runs through jax/PJRT on the neuron (axon) platform; a
# cpu-pinned JAX_PLATFORMS would hide the NeuronCores.
if os.environ.get("JAX_PLATFORMS") == "cpu":
    os.environ["JAX_PLATFORMS"] = ""

for _p in ("/opt/trn_rl_repo",):
    if _p not in sys.path:
        sys.path.insert(0, _p)

import numpy as np
from contextlib import ExitStack

import concourse.bacc as bacc
import concourse.mybir as mybir
from concourse import tile
from concourse import bass_utils

B, N, D_IN, D_H = 65536, 8, 256, 128
SLOPE = 0.3
CORES = 8
RC = B * N // CORES          # rows per core (65536)
BC = B // CORES              # batches per core (8192)
BLK = 512                    # rows per block
NBLK = RC // BLK             # 128
NG = BLK // N                # batch groups per partition in the tail (64)

f32 = mybir.dt.float32
f32r = mybir.dt.float32r
AF = mybir.ActivationFunctionType
ALU = mybir.AluOpType

_NC_CACHE = []
_LAST_RESULTS = None


def _build():
    nc = bacc.Bacc(
        "TRN2", target_bir_lowering=False, debug=False, num_devices=CORES
    )
    xt_d = nc.dram_tensor("xt", [D_IN, RC], f32r, kind="ExternalInput").ap()
    w1_d = nc.dram_tensor("w1", [D_IN, D_H], f32r, kind="ExternalInput").ap()
    w2_d = nc.dram_tensor("w2", [D_H, 1], f32r, kind="ExternalInput").ap()
    bias7_d = nc.dram_tensor("bias7", [D_H, 1], f32, kind="ExternalInput").ap()
    st_d = nc.dram_tensor("st", [128, 1], f32, kind="ExternalInput").ap()
    nst_d = nc.dram_tensor("nst", [128, 1], f32, kind="ExternalInput").ap()
    p_d = nc.dram_tensor("p", [BC, N + 1], f32, kind="ExternalOutput").ap()

    with tile.TileContext(nc) as tc, ExitStack() as ctx:
        const = ctx.enter_context(tc.tile_pool(name="const", bufs=1))
        xpool = ctx.enter_context(tc.tile_pool(name="xp", bufs=1))
        hpool = ctx.enter_context(tc.tile_pool(name="hp", bufs=1))
        bpool = ctx.enter_context(tc.tile_pool(name="bp", bufs=1))
        tpool = ctx.enter_context(tc.tile_pool(name="tp", bufs=1))
        psh = ctx.enter_context(tc.tile_pool(name="psh", bufs=1, space="PSUM"))
        psb = ctx.enter_context(tc.tile_pool(name="psb", bufs=1, space="PSUM"))

        def T(pool, shape, dt_, nm, bufs=1):
            tag = nm.split("_")[0]
            return pool.tile(shape, dt_, name=nm, tag=tag, bufs=bufs)

        GRP = 8                  # compute blocks per DMA / staging group
        DBLK = GRP * BLK         # 4096 cols, 16 KB per partition per chunk

        w1_sb = T(const, [128, 2, D_H], f32r, "w1sb")
        nc.sync.dma_start(w1_sb[:], w1_d.rearrange("(kc p) m -> p kc m", kc=2))
        w2_sb = T(const, [D_H, 1], f32r, "w2sb")
        nc.sync.dma_start(w2_sb[:], w2_d[:])
        bias7_sb = T(const, [D_H, 1], f32, "bias7sb")
        nc.sync.dma_start(bias7_sb[:], bias7_d[:])
        st_sb = T(const, [128, 1], f32, "stsb")
        nc.sync.dma_start(st_sb[:], st_d[:])
        nst_sb = T(const, [128, 1], f32, "nstsb")
        nc.sync.dma_start(nst_sb[:], nst_d[:])

        # beta accumulator: partition = block index, free = row-in-block
        bt = T(bpool, [128, BLK], f32, "bt")

        for dblk in range(NBLK // GRP):
            x0 = T(xpool, [128, DBLK], f32r, f"x0_{dblk}", bufs=4)
            nc.sync.dma_start(x0[:], xt_d[0:128, dblk * DBLK:(dblk + 1) * DBLK])
            x1 = T(xpool, [128, DBLK], f32r, f"x1_{dblk}", bufs=4)
            nc.sync.dma_start(x1[:], xt_d[128:256, dblk * DBLK:(dblk + 1) * DBLK])
            bs = T(bpool, [1, DBLK], f32, f"bs_{dblk}", bufs=3)
            for sub in range(GRP):
                blk = dblk * GRP + sub
                cs = slice(sub * BLK, (sub + 1) * BLK)

                ph = T(psh, [128, BLK], f32, f"ph_{blk}", bufs=4)
                nc.tensor.matmul(ph[:], w1_sb[:, 0, :], x0[:, cs], start=True, stop=False)
                nc.tensor.matmul(ph[:], w1_sb[:, 1, :], x1[:, cs], start=False, stop=True)

                # leaky_relu(z + b1) = 0.3*(z + b1) + 0.7*relu(z + b1)
                #   r  = relu(0.7*z + 0.7*b1)              (ACT)
                #   hh = 0.3*z + r                          (DVE; 0.3*b1 in st)
                r_sb = T(hpool, [128, BLK], f32, f"r_{blk}", bufs=4)
                nc.scalar.activation(
                    r_sb[:], ph[:], AF.Relu, bias=bias7_sb[:], scale=0.7
                )
                hh = T(hpool, [128, BLK], f32r, f"hh_{blk}", bufs=4)
                nc.vector.scalar_tensor_tensor(
                    hh[:], ph[:], SLOPE, r_sb[:], op0=ALU.mult, op1=ALU.add
                )

                pb = T(psb, [1, BLK], f32, f"pb_{blk}", bufs=4)
                nc.tensor.matmul(pb[:], w2_sb[:], hh[:], start=True, stop=True)
                # PSUM -> SBUF staging of beta_pre rows: 1-lane copies,
                # split between DVE and ACT so neither chokes.
                if blk % 2 == 0:
                    nc.vector.tensor_copy(bs[0:1, cs], pb[:])
                else:
                    nc.scalar.activation(bs[0:1, cs], pb[:], AF.Copy)
            # one fan-out DMA redistributes GRP beta rows to partition-per-block
            nc.scalar.dma_start(
                bt[dblk * GRP:(dblk + 1) * GRP, :],
                bs[:].rearrange("p (j r) -> p j r", j=GRP),
            )

        # ---- tail: stick-breaking over the N axis (groups of 8 in free dim)
        sg = T(tpool, [128, BLK], f32, "sg")
        nc.scalar.activation(sg[:], bt[:], AF.Sigmoid, bias=st_sb[:], scale=1.0)
        g = T(tpool, [128, BLK], f32, "g")  # 1 - beta = sigmoid(-(x + st))
        nc.scalar.activation(g[:], bt[:], AF.Sigmoid, bias=nst_sb[:], scale=-1.0)

        # suffix products s[e] = prod_{k>=e} g[k] via in-place log-tree:
        # s[0:N-k] *= s[k:N] reads ahead of writes (forward refs are safe)
        s = T(tpool, [128, BLK], f32, "s")
        nc.vector.tensor_copy(s[:], g[:])
        sv = s[:].rearrange("p (gr e) -> p gr e", e=N)
        for k in (1, 2, 4):
            nc.vector.tensor_mul(sv[:, :, 0:N - k], sv[:, :, 0:N - k], sv[:, :, k:N])

        # P[gr*9]     = s[gr*8]                   (p[b, 0])
        # P[gr*9 + i] = beta[i-1] * s[i], i=1..7  (s[8] == 1 -> P[..,8]=beta[7])
        P = T(tpool, [128, NG * (N + 1)], f32, "P")
        Pv = P[:].rearrange("p (gr e) -> p gr e", e=N + 1)
        sgv = sg[:].rearrange("p (gr e) -> p gr e", e=N)
        nc.vector.tensor_copy(Pv[:, :, 0:1], sv[:, :, 0:1])
        nc.vector.tensor_mul(Pv[:, :, 1:N], sgv[:, :, 0:N - 1], sv[:, :, 1:N])
        nc.vector.tensor_copy(Pv[:, :, N:N + 1], sgv[:, :, N - 1:N])
        nc.sync.dma_start(
            p_d.rearrange("(blk gr) e -> blk (gr e)", gr=NG), P[:]
        )

    nc.compile()
    return nc


def _get_nc():
    if not _NC_CACHE:
        _NC_CACHE.append(_build())
    return _NC_CACHE[0]


def kernel(**inputs):
    x = np.asarray(inputs["x"], dtype=np.float32)
    W1 = np.ascontiguousarray(np.asarray(inputs["W1"], dtype=np.float32))
    b1 = np.asarray(inputs["b1"], dtype=np.float32)
    W2 = np.ascontiguousarray(np.asarray(inputs["W2"], dtype=np.float32))
    b2 = np.asarray(inputs["b2"], dtype=np.float32)

    nc = _get_nc()

    xf = x.reshape(B * N, D_IN)
    st_val = np.float32(float(b2[0]) + SLOPE * float(b1 @ W2[:, 0]))
    bias7 = np.ascontiguousarray((0.7 * b1).reshape(D_H, 1).astype(np.float32))
    stv = np.full((128, 1), st_val, np.float32)
    nstv = np.ascontiguousarray(-stv)

    in_maps = []
    for c in range(CORES):
        shard = xf[c * RC:(c + 1) * RC]
        xt = np.ascontiguousarray(shard.T)   # [256, RC]
        in_maps.append({
            "xt": xt, "w1": W1, "w2": W2,
            "bias7": bias7, "st": stv, "nst": nstv,
        })

    res = bass_utils.run_bass_kernel_spmd(
        nc, in_maps, core_ids=list(range(CORES))
    )
    global _LAST_RESULTS
    _LAST_RESULTS = res
    p = np.concatenate(
        [res.results[c]["p"] for c in range(CORES)], axis=0
    ).astype(np.float32)
    return p



# revision 8
# speedup vs baseline: 1.7483x; 1.7483x over previous
"""Trainium2 Bass kernel for nn_Distribution_74758200754679.

Computes, for x [65536, 8, 256] and a tiny MLP (256 -> 128 -> 1):
    h    = leaky_relu(x @ W1 + b1, 0.3)
    beta = sigmoid(h @ W2 + b2)            # [B, N]
    p    = stick_breaking(beta)            # [B, N+1]

Distribution: pure data parallel over 8 NeuronCores — x is sharded along
the batch axis, MLP params are replicated. Each core's shard is staged
host-side in transposed fp16 layout (d_in on partitions), halving HBM
traffic vs fp32 and enabling fast weight load on the PE.

Per-core device program (32 MB of x per core, 128 blocks x 512 rows):
  DMA xT chunks -> PE fp16 matmuls (K=256 accumulated in PSUM, 1024-wide
  tiles) -> leaky split as ACT r=relu(0.7z+0.7b1) + DVE hh=0.3z+r (the
  0.3*b1@W2 deficit is folded into the sigmoid shift host-side)
  -> col-tiled L2 matmuls (4x M=1 packed into PE column groups, outputs
  on PSUM partitions 0/32/64/96) -> DMA fan-out straight from PSUM into
  the [block, row] beta tile -> per-half tail: sigmoid + suffix-product
  stick-breaking, DMA out.
"""

import os
import sys

# The device path runs through jax/PJRT on the neuron (axon) platform; a
# cpu-pinned JAX_PLATFORMS would hide the NeuronCores.
if os.environ.get("JAX_PLATFORMS") == "cpu":
    os.environ["JAX_PLATFORMS"] = ""

for _p in ("/opt/trn_rl_repo",):
    if _p not in sys.path:
        sys.path.insert(0, _p)

import numpy as np
from contextlib import ExitStack

import concourse.bacc as bacc
import concourse.mybir as mybir
from concourse import tile
from concourse import bass_utils

B, N, D_IN, D_H = 65536, 8, 256, 128
SLOPE = 0.3
CORES = 8
RC = B * N // CORES          # rows per core (65536)
BC = B // CORES              # batches per core (8192)
BLK = 512                    # rows per block
NBLK = RC // BLK             # 128
NG = BLK // N                # batch groups per partition in the tail (64)

f32 = mybir.dt.float32
f16 = mybir.dt.float16
AF = mybir.ActivationFunctionType
ALU = mybir.AluOpType

_NC_CACHE = []
_LAST_RESULTS = None

GRP = 8                  # compute blocks per DMA / fan-out group
DBLK = GRP * BLK         # 4096 cols, 8 KB (fp16) per partition/chunk
NDBLK = NBLK // GRP      # 16


def _build():
    nc = bacc.Bacc(
        "TRN2", target_bir_lowering=False, debug=False, num_devices=CORES
    )
    xt_d = nc.dram_tensor("xt", [D_IN, RC], f16, kind="ExternalInput").ap()
    w1_d = nc.dram_tensor("w1", [D_IN, D_H], f16, kind="ExternalInput").ap()
    w2_d = nc.dram_tensor("w2", [D_H, 1], f16, kind="ExternalInput").ap()
    # cb columns: 0 = 0.7*b1, 1 = st (sigmoid shift), 2 = -st
    cb_d = nc.dram_tensor("cb", [D_H, 3], f32, kind="ExternalInput").ap()
    p_d = nc.dram_tensor("p", [BC, N + 1], f32, kind="ExternalOutput").ap()

    with tile.TileContext(nc) as tc, ExitStack() as ctx:
        const = ctx.enter_context(tc.tile_pool(name="const", bufs=1))
        xpool = ctx.enter_context(tc.tile_pool(name="xp", bufs=1))
        hpool = ctx.enter_context(tc.tile_pool(name="hp", bufs=1))
        bpool = ctx.enter_context(tc.tile_pool(name="bp", bufs=1))
        tpool = ctx.enter_context(tc.tile_pool(name="tp", bufs=1))
        psh = ctx.enter_context(tc.tile_pool(name="psh", bufs=1, space="PSUM"))
        psb = ctx.enter_context(tc.tile_pool(name="psb", bufs=1, space="PSUM"))

        def T(pool, shape, dt_, nm, bufs=1):
            tag = nm.split("_")[0]
            return pool.tile(shape, dt_, name=nm, tag=tag, bufs=bufs)

        # first x chunk ahead of everything on the sync ring
        x0s, x1s = [], []

        def load_x(dblk):
            x0 = T(xpool, [128, DBLK], f16, f"x0_{dblk}", bufs=6)
            nc.sync.dma_start(x0[:], xt_d[0:128, dblk * DBLK:(dblk + 1) * DBLK])
            x1 = T(xpool, [128, DBLK], f16, f"x1_{dblk}", bufs=6)
            nc.sync.dma_start(x1[:], xt_d[128:256, dblk * DBLK:(dblk + 1) * DBLK])
            x0s.append(x0)
            x1s.append(x1)

        load_x(0)

        # tiny consts ride the (otherwise idle at startup) scalar ring
        w1_sb = T(const, [128, 2, D_H], f16, "w1sb")
        nc.scalar.dma_start(w1_sb[:], w1_d.rearrange("(kc p) m -> p kc m", kc=2))
        w2_sb = T(const, [D_H, 1], f16, "w2sb")
        nc.scalar.dma_start(w2_sb[:], w2_d[:])
        cb_sb = T(const, [D_H, 3], f32, "cbsb")
        nc.scalar.dma_start(cb_sb[:], cb_d[:])
        b7_ap = cb_sb[:, 0:1]
        st_ap = cb_sb[:, 1:2]
        nst_ap = cb_sb[:, 2:3]

        # beta accumulator: partition = block index, free = row-in-block
        bt = T(bpool, [128, BLK], f32, "bt")

        def tail_half(h):
            # stick-breaking over the N axis for partitions (blocks)
            # h*64 .. h*64+63; groups of N=8 along the free dim.
            rows = slice(h * 64, (h + 1) * 64)
            sg = T(tpool, [128, BLK], f32, f"sg_{h}")
            nc.scalar.activation(
                sg[rows, :], bt[rows, :], AF.Sigmoid, bias=st_ap[0:64], scale=1.0
            )
            g = T(tpool, [128, BLK], f32, f"g_{h}")  # 1-beta = sigmoid(-(x+st))
            nc.scalar.activation(
                g[rows, :], bt[rows, :], AF.Sigmoid, bias=nst_ap[0:64], scale=-1.0
            )
            # suffix products s[e] = prod_{k>=e} g[k] via in-place log-tree:
            # s[0:N-k] *= s[k:N] reads ahead of writes (forward refs are safe)
            s = T(tpool, [128, BLK], f32, f"s_{h}")
            nc.vector.tensor_copy(s[rows, :], g[rows, :])
            sv = s[:].rearrange("p (gr e) -> p gr e", e=N)
            for k in (1, 2, 4):
                nc.vector.tensor_mul(
                    sv[rows, :, 0:N - k], sv[rows, :, 0:N - k], sv[rows, :, k:N]
                )
            # P[gr*9]     = s[gr*8]                   (p[b, 0])
            # P[gr*9 + i] = beta[i-1]*s[i], i=1..7   (s[8]==1 -> P[..,8]=beta[7])
            P = T(tpool, [128, NG * (N + 1)], f32, f"P_{h}")
            Pv = P[:].rearrange("p (gr e) -> p gr e", e=N + 1)
            sgv = sg[:].rearrange("p (gr e) -> p gr e", e=N)
            nc.vector.tensor_copy(Pv[rows, :, 0:1], sv[rows, :, 0:1])
            nc.vector.tensor_mul(
                Pv[rows, :, 1:N], sgv[rows, :, 0:N - 1], sv[rows, :, 1:N]
            )
            nc.vector.tensor_copy(Pv[rows, :, N:N + 1], sgv[rows, :, N - 1:N])
            nc.scalar.dma_start(
                p_d.rearrange("(blk gr) e -> blk (gr e)", gr=NG)[rows, :],
                P[rows, :],
            )

        # per-dblk state for the one-dblk L2/fan-out pipeline lag
        hh_by_dblk = {}

        def l2_and_fanout(d):
            """Col-tiled L2 matmuls + PSUM->SBUF fan-out DMA for dblk d."""
            hhs = hh_by_dblk.pop(d)
            for gg in range(2):        # two groups of 4 blocks
                pb4 = T(psb, [128, BLK], f32, f"pb4_{d}_{gg}", bufs=2)
                for j in range(4):     # in-group block j -> col group j
                    hh = hhs[2 * gg + j // 2]
                    half = slice((j % 2) * BLK, (j % 2 + 1) * BLK)
                    nc.tensor.matmul(
                        pb4[32 * j:32 * j + 1, :], w2_sb[:], hh[:, half],
                        start=True, stop=True, tile_position=(0, 32 * j),
                    )
                # staging copy PSUM -> SBUF: engines need partition stride 1,
                # so copy the whole 0..96 partition range (engine time is
                # free-dim-bound; extra partitions are free) and let the
                # fan-out DMA pick rows {0,32,64,96}. Alternate engines so
                # neither ACT nor DVE eats the whole staging cost.
                bs97 = T(bpool, [97, BLK], f32, f"bs_{d}_{gg}", bufs=4)
                if gg == 0:
                    nc.scalar.activation(bs97[:], pb4[0:97, :], AF.Copy)
                else:
                    nc.vector.tensor_copy(bs97[:], pb4[0:97, :])
                base = d * GRP + gg * 4
                nc.scalar.dma_start(bt[base:base + 4, :], bs97[0:97:32, :])

        for dblk in range(NDBLK):
            if dblk + 1 < NDBLK:
                load_x(dblk + 1)
            x0, x1 = x0s[dblk], x1s[dblk]
            hhs = []
            for sub in range(4):       # 1024 cols (2 blocks) per sub
                c0 = slice(sub * 1024, sub * 1024 + 512)
                c1 = slice(sub * 1024 + 512, sub * 1024 + 1024)
                ph = T(psh, [128, 1024], f32, f"ph_{dblk}_{sub}", bufs=3)
                nc.tensor.matmul(ph[:, 0:512], w1_sb[:, 0, :], x0[:, c0], start=True, stop=False)
                nc.tensor.matmul(ph[:, 512:1024], w1_sb[:, 0, :], x0[:, c1], start=True, stop=False)
                nc.tensor.matmul(ph[:, 0:512], w1_sb[:, 1, :], x1[:, c0], start=False, stop=True)
                nc.tensor.matmul(ph[:, 512:1024], w1_sb[:, 1, :], x1[:, c1], start=False, stop=True)

                # leaky(z+b1) - 0.3*b1 = 0.3*z + relu(0.7*z + 0.7*b1);
                # the 0.3*b1@W2 deficit is folded into st host-side.
                r = T(hpool, [128, 1024], f16, f"r_{dblk}_{sub}", bufs=3)
                nc.scalar.activation(r[:], ph[:], AF.Relu, bias=b7_ap, scale=0.7)
                hh = T(hpool, [128, 1024], f16, f"hh_{dblk}_{sub}", bufs=8)
                nc.vector.scalar_tensor_tensor(
                    hh[:], ph[:], SLOPE, r[:], op0=ALU.mult, op1=ALU.add
                )
                hhs.append(hh)
            hh_by_dblk[dblk] = hhs
            if dblk > 0:
                l2_and_fanout(dblk - 1)
            if dblk == NDBLK // 2:
                tail_half(0)
        l2_and_fanout(NDBLK - 1)
        tail_half(1)

    nc.compile()
    return nc


def _get_nc(*_a):
    if not _NC_CACHE:
        _NC_CACHE.append(_build())
    return _NC_CACHE[0]


def kernel(**inputs):
    x = np.asarray(inputs["x"], dtype=np.float32)
    W1 = np.ascontiguousarray(np.asarray(inputs["W1"], dtype=np.float32))
    b1 = np.asarray(inputs["b1"], dtype=np.float32)
    W2 = np.ascontiguousarray(np.asarray(inputs["W2"], dtype=np.float32))
    b2 = np.asarray(inputs["b2"], dtype=np.float32)

    nc = _get_nc()

    xf = x.reshape(B * N, D_IN)
    st_val = np.float32(float(b2[0]) + SLOPE * float(b1 @ W2[:, 0]))
    cb = np.zeros((D_H, 3), np.float32)
    cb[:, 0] = 0.7 * b1
    cb[:, 1] = st_val
    cb[:, 2] = -st_val

    w1h = np.ascontiguousarray(W1.astype(np.float16))
    w2h = np.ascontiguousarray(W2.astype(np.float16))

    in_maps = []
    for c in range(CORES):
        shard = xf[c * RC:(c + 1) * RC]
        xt = np.ascontiguousarray(shard.T.astype(np.float16))   # [256, RC]
        in_maps.append({
            "xt": xt, "w1": w1h, "w2": w2h, "cb": cb,
        })

    res = bass_utils.run_bass_kernel_spmd(
        nc, in_maps, core_ids=list(range(CORES))
    )
    global _LAST_RESULTS
    _LAST_RESULTS = res
    p = np.concatenate(
        [res.results[c]["p"] for c in range(CORES)], axis=0
    ).astype(np.float32)
    return p
